# revision 21
# baseline (speedup 1.0000x reference)
"""Trainium2 Bass kernel for nn_CnnModel_70007966925195.

CNN backbone (3x conv1d+relu+maxpool2 -> mean -> FC+relu -> BN) followed by an
all-pairs contrastive loss. Data-parallel over N across 8 NeuronCores; z is
AllGathered (fp16) and each core computes a 512x4096 row block of the loss.

Layout strategy:
- conv1 (C_in=1, k=100): fp16. x stored transposed+padded as overlapping
  128-row position chunks in SBUF; the tap-window select is folded into
  pre-shifted weight matrices (zero-padded to K=128). Two output positions
  (l, l+2) are packed into one M=128 matmul (cols 0-63 / 64-127), free dim =
  all 512 local samples.
- conv2/conv3: fp8(e4m3) activations+weights with DoubleRow matmuls (K=256
  per instruction: two 128-row K-halves read from adjacent ring slots).
  conv2: per output position one DR matmul (4 taps) + one K=64 leftover tap;
  the even/odd leftovers sit in disjoint row groups (0-63 / 64-127) and run
  concurrently via tile_position. conv3: DR (taps 0,1) + K=128 leftover
  (tap 2) per out-channel half. Rings carry 9 slots: slot 8 duplicates slot
  0 so DR pairs (7,0) stay contiguous. relu+maxpool eviction = ACT
  relu(bank0) -> SBUF f32, then DVE max(tmp, bank1) -> fp8 ring.
- mean+FC: h3 tiles are summed into an SBUF accumulator on the otherwise
  idle Pool engine (weights pre-scaled by 1/64); FC itself is 2 matmuls.
- loss: z AllGathered as fp16 (local rows stay f32); d2 per 128x512 tile is
  one fp32r K=128 matmul (z.z) plus one fp32r K=2 rank-2 matmul (sq_i+sq_j);
  y-mask via bf16 K=2 matmuls precomputed during the collective; then
  clamp/sqrt/relu(1-d)/select, DMA out. Pairwise distances are translation
  invariant, so rows AND gathered columns are centered by the local mean
  embedding to kill catastrophic cancellation.
"""

import os
import sys

try:
    import concourse.bass as bass  # noqa: F401
except ImportError:
    sys.path.insert(0, "/opt/trn_rl_repo")

import numpy as np

import concourse.bass as bass  # noqa: F811
import concourse.mybir as mybir
import concourse.tile as tile
from concourse import bacc
from concourse.bass_utils import run_bass_kernel_spmd

F32 = mybir.dt.float32
F32R = mybir.dt.float32r
BF16 = mybir.dt.bfloat16
F16 = mybir.dt.float16
FP8 = mybir.dt.float8e4
AL = mybir.AluOpType
ACT = mybir.ActivationFunctionType
DR = mybir.MatmulPerfMode.DoubleRow

N_CORES = 8
N = 4096
NL = N // N_CORES   # 512 samples per core
L = 512
K1, C1 = 100, 64          # conv1 kernel/outch
K2, C2 = 5, 128           # conv2
K3, C3 = 3, 256           # conv3
NCHUNK1 = 18              # conv1 x chunks, stride 29
SIG = 29                  # shift count (chunk stride)
T1 = 256                  # pooled conv1 positions
T2 = 128                  # pooled conv2 positions
T3 = 64                   # pooled conv3 positions
W1R = 8                   # h1 ring depth (slot 8 = dup of slot 0)
W2R = 8                   # h2 ring depth (slot 8 = dup of slot 0)

LAST_RESULT = None        # BassKernelResults stash for test harness


def build_nc():
    kdebug = os.environ.get("KDEBUG", "full")
    nc = bacc.Bacc("TRN2", target_bir_lowering=False, debug=False,
                   num_devices=N_CORES)

    xs_d = nc.dram_tensor("xs", [NCHUNK1, 128, NL], F16, kind="ExternalInput")
    w1s_d = nc.dram_tensor("w1s", [31, 128, 128], F16, kind="ExternalInput")
    w2e_d = nc.dram_tensor("w2e", [2, 128, C2], FP8, kind="ExternalInput")
    w2o_d = nc.dram_tensor("w2o", [2, 128, C2], FP8, kind="ExternalInput")
    w2l_d = nc.dram_tensor("w2l", [128, C2], FP8, kind="ExternalInput")
    w3p_d = nc.dram_tensor("w3p", [2, 128, C3], FP8, kind="ExternalInput")
    w3l_d = nc.dram_tensor("w3l", [128, C3], FP8, kind="ExternalInput")
    fcw_d = nc.dram_tensor("fcw", [2, 128, 128], F32R, kind="ExternalInput")
    fcb_d = nc.dram_tensor("fcb", [128, 1], F32, kind="ExternalInput")
    bna_d = nc.dram_tensor("bna", [128, 1], F32, kind="ExternalInput")
    bnb_d = nc.dram_tensor("bnb", [128, 1], F32, kind="ExternalInput")
    abl_d = nc.dram_tensor("abl", [2, NL], BF16, kind="ExternalInput")
    abf_d = nc.dram_tensor("abf", [2, N], BF16, kind="ExternalInput")
    onc_d = nc.dram_tensor("onc", [128, 1], F32R, kind="ExternalInput")
    onr_d = nc.dram_tensor("onr", [1, N], F32R, kind="ExternalInput")
    out_d = nc.dram_tensor("out", [NL, N], F16, kind="ExternalOutput")
    gin_d = nc.dram_tensor("gin", [128, NL], F16, kind="Internal")
    gout_d = nc.dram_tensor("gout", [N_CORES, 128, NL], F16, kind="Internal",
                            addr_space="Shared")

    with tile.TileContext(nc) as tc:
        with (
            tc.tile_pool(name="const", bufs=1) as cpool,
            tc.tile_pool(name="zbuf", bufs=1) as zpool,
        ):
            # ---- persistent SBUF tensors (issue order = startup order) ----
            w1s = cpool.tile([128, 31, 128], F16, tag="w1s")
            nc.sync.dma_start(w1s[:, 0:2, :],
                              w1s_d[0:2].rearrange("s k o -> k s o"))
            xs = cpool.tile([128, NCHUNK1, NL], F16, tag="xs")
            nc.sync.dma_start(xs[:, 0, :], xs_d[0, :, :])
            nc.sync.dma_start(w1s[:, 2:8, :],
                              w1s_d[2:8].rearrange("s k o -> k s o"))
            nc.sync.dma_start(xs[:, 1, :], xs_d[1, :, :])
            nc.sync.dma_start(w1s[:, 8:31, :],
                              w1s_d[8:31].rearrange("s k o -> k s o"))
            nc.sync.dma_start(xs[:, 2:4, :],
                              xs_d[2:4].rearrange("c k n -> k c n"))
            nc.sync.dma_start(xs[:, 4:6, :],
                              xs_d[4:6].rearrange("c k n -> k c n"))
            w2e = cpool.tile([128, 2, C2], FP8, tag="w2e")
            nc.sync.dma_start(w2e[:], w2e_d[:].rearrange("t k o -> k t o"))
            w2o = cpool.tile([128, 2, C2], FP8, tag="w2o")
            nc.sync.dma_start(w2o[:], w2o_d[:].rearrange("t k o -> k t o"))
            w2l = cpool.tile([128, C2], FP8, tag="w2l")
            nc.sync.dma_start(w2l[:], w2l_d[:])
            nc.sync.dma_start(xs[:, 6:12, :],
                              xs_d[6:12].rearrange("c k n -> k c n"))
            w3p = cpool.tile([128, 2, C3], FP8, tag="w3p")
            nc.sync.dma_start(w3p[:], w3p_d[:].rearrange("t k o -> k t o"))
            w3l = cpool.tile([128, C3], FP8, tag="w3l")
            nc.sync.dma_start(w3l[:], w3l_d[:])
            nc.sync.dma_start(xs[:, 12:18, :],
                              xs_d[12:18].rearrange("c k n -> k c n"))
            fcw = cpool.tile([128, 2, 128], F32R, tag="fcw")
            nc.sync.dma_start(fcw[:], fcw_d[:].rearrange("c k o -> k c o"))
            fcb = cpool.tile([128, 1], F32, tag="fcb")
            nc.sync.dma_start(fcb[:], fcb_d[:])
            bna = cpool.tile([128, 1], F32, tag="bna")
            nc.sync.dma_start(bna[:], bna_d[:])
            bnb = cpool.tile([128, 1], F32, tag="bnb")
            nc.sync.dma_start(bnb[:], bnb_d[:])
            abl = cpool.tile([2, NL], BF16, tag="abl")
            nc.sync.dma_start(abl[:], abl_d[:])
            abf = cpool.tile([2, N], BF16, tag="abf")
            nc.sync.dma_start(abf[:], abf_d[:])

            h1r = cpool.tile([128, W1R + 1, NL], FP8, tag="h1r")
            h2r = cpool.tile([128, W2R + 1, NL], FP8, tag="h2r")
            hsum = cpool.tile([128, 2, NL], F32, tag="hsum")
            nc.gpsimd.memset(hsum[:], 0.0)

            # ---- fused conv pipeline ----
            with (
                tc.tile_pool(name="p1", bufs=1, space="PSUM") as p1pool,
                tc.tile_pool(name="p2", bufs=1, space="PSUM") as p2pool,
                tc.tile_pool(name="p3", bufs=2, space="PSUM") as p3pool,
                tc.tile_pool(name="h3", bufs=3) as h3pool,
                tc.tile_pool(name="ev", bufs=4) as evpool,
            ):
                def conv1_batch(i1):
                    # positions l = 4*i1 .. 4*i1+3 -> pooled t = 2*i1, 2*i1+1
                    # bank j: partitions 0-63 = pos 4i+j, 64-127 = pos 4i+2+j
                    ps = p1pool.tile([128, 2, NL], F32, tag="p1")
                    for j in range(2):
                        la = 4 * i1 + j
                        lb = la + 2
                        ca, sa = divmod(la, SIG)
                        cb, sb_ = divmod(lb, SIG)
                        if ca == cb:
                            nc.tensor.matmul(
                                ps[:, j, :], w1s[:, sa, :], xs[:, ca, :],
                                start=True, stop=True)
                        else:
                            nc.tensor.matmul(
                                ps[:, j, :], w1s[:, sa, :], xs[:, ca, :],
                                start=True, stop=False)
                            nc.tensor.matmul(
                                ps[:, j, :], w1s[:, 29 + sb_, :],
                                xs[:, cb, :],
                                start=False, stop=True)
                    # eviction: DVE may read only ONE PSUM operand per op,
                    # so ACT moves relu(bank0) to SBUF first (max(relu(a),b)
                    # == relu(max(a,b)) since b is max'd with a relu output)
                    tmp = evpool.tile([128, NL], F32, tag="ev")
                    nc.scalar.activation(tmp[:], ps[:, 0, :], ACT.Relu)
                    slot = i1 % W1R
                    nc.vector.tensor_max(h1r[:, slot, :], tmp[:],
                                         ps[:, 1, :])
                    if slot == 0:
                        nc.vector.tensor_copy(h1r[:, W1R, :], h1r[:, 0, :])

                def conv2_pair(j2):
                    # output positions l2 = 2*j2 (bank0), 2*j2+1 (bank1).
                    # even: DR pair = h1 slots (j2-1, j2) with [t0;t1],[t2;t3]
                    #       leftover = tap t4 (rows 0-63) @ slot j2+1
                    # odd:  DR pair = slots (j2, j2+1) with [t1;t2],[t3;t4]
                    #       leftover = tap t0 (rows 64-127) @ slot j2-1
                    # Each bank's matmul list is built first so start/stop
                    # land on the true first/last op of the group; the two
                    # leftovers occupy disjoint row groups (0-63 / 64-127)
                    # and are emitted adjacently so they run concurrently.
                    ps = p2pool.tile([128, 2, NL], F32, tag="p2")
                    main, left = [], []
                    if j2 >= 1:
                        main.append((0, "dr", w2e[:], (j2 - 1) % W1R))
                    else:        # l2 = 0: only taps t2,t3 @ slot 0
                        main.append((0, "n", w2e[:, 1, :], 0))
                    if j2 + 1 < T1 // 2:
                        left.append((0, 0, (j2 + 1) % W1R))
                    if j2 + 1 < T1 // 2:
                        main.append((1, "dr", w2o[:], j2 % W1R))
                    else:        # l2 = 255: only taps t1,t2 @ slot 127
                        main.append((1, "n", w2o[:, 0, :], j2 % W1R))
                    if j2 >= 1:
                        left.append((1, 64, (j2 - 1) % W1R))
                    has_left = {b for b, _, _ in left}
                    for bank, kind, w, slot in main:
                        stop = bank not in has_left
                        if kind == "dr":
                            nc.tensor.matmul(ps[:, bank, :], w,
                                             h1r[:, slot:slot + 2, :],
                                             start=True, stop=stop,
                                             perf_mode=DR)
                        else:
                            nc.tensor.matmul(ps[:, bank, :], w,
                                             h1r[:, slot, :],
                                             start=True, stop=stop)
                    for bank, rb, slot in left:
                        nc.tensor.matmul(
                            ps[:, bank, :], w2l[rb:rb + 64, :],
                            h1r[rb:rb + 64, slot, :],
                            start=False, stop=True,
                            tile_position=(rb, 0))
                    tmp = evpool.tile([128, NL], F32, tag="ev")
                    nc.scalar.activation(tmp[:], ps[:, 0, :], ACT.Relu)
                    slot = j2 % W2R
                    nc.vector.tensor_max(h2r[:, slot, :], tmp[:],
                                         ps[:, 1, :])
                    if slot == 0:
                        nc.vector.tensor_copy(h2r[:, W2R, :], h2r[:, 0, :])

                def conv3_half(j3, ch):
                    # output positions l3 = 2*j3 (bank0), 2*j3+1 (bank1),
                    # out-channel half ch.  DR pair = h2 slots (l3-1, l3)
                    # with taps (t0, t1); leftover = tap t2 @ slot l3+1.
                    ps = p3pool.tile([128, 2, NL], F32, tag="p3")
                    cs = slice(128 * ch, 128 * ch + 128)
                    for jj in range(2):
                        l3 = 2 * j3 + jj
                        has_left = l3 + 1 < T2
                        if l3 >= 1:
                            a = (l3 - 1) % W2R
                            nc.tensor.matmul(ps[:, jj, :], w3p[:, :, cs],
                                             h2r[:, a:a + 2, :],
                                             start=True, stop=not has_left,
                                             perf_mode=DR)
                        else:
                            # l3 = 0: tap t1 @ slot 0 only
                            nc.tensor.matmul(ps[:, jj, :], w3p[:, 1, cs],
                                             h2r[:, 0, :],
                                             start=True, stop=not has_left)
                        if has_left:
                            nc.tensor.matmul(ps[:, jj, :], w3l[:, cs],
                                             h2r[:, (l3 + 1) % W2R, :],
                                             start=False, stop=True)
                    h3t = h3pool.tile([128, NL], F16, tag="h3t")
                    tmp = evpool.tile([128, NL], F32, tag="ev")
                    nc.scalar.activation(tmp[:], ps[:, 0, :], ACT.Relu)
                    nc.vector.tensor_max(h3t[:], tmp[:], ps[:, 1, :])
                    nc.gpsimd.tensor_tensor(hsum[:, ch, :], hsum[:, ch, :],
                                            h3t[:], AL.add)

                for ii in range(132):
                    if ii < 128:
                        conv1_batch(ii)
                    if 2 <= ii < 130:
                        conv2_pair(ii - 2)
                    if ii >= 4:
                        j3, ch = divmod(ii - 4, 2)
                        if j3 < T3:
                            conv3_half(j3, ch)

            _emit_tail(nc, tc, zpool, hsum, fcw, fcb, bna, bnb, abl, abf,
                       onc_d, onr_d, out_d, gin_d, gout_d, kdebug)

    nc.compile()
    return nc


def _emit_tail(nc, tc, zpool, hsum, fcw, fcb, bna, bnb, abl, abf,
               onc_d, onr_d, out_d, gin_d, gout_d, kdebug):
    # ---- FC from pooled means, z = BN(relu(FC)) ; gather z + |z|^2 ----
    with (
        tc.tile_pool(name="fcp", bufs=1, space="PSUM") as fcpool,
        tc.tile_pool(name="sqp", bufs=1, space="PSUM") as sqpool,
    ):
        hsb = zpool.tile([128, 2, NL], F32R, tag="hsb")
        nc.vector.tensor_copy(hsb[:], hsum[:])
        fc_ps = fcpool.tile([128, NL], F32, tag="fc")
        nc.tensor.matmul(fc_ps[:], fcw[:, 0, :], hsb[:, 0, :],
                         start=True, stop=False)
        nc.tensor.matmul(fc_ps[:], fcw[:, 1, :], hsb[:, 1, :],
                         start=False, stop=True)

        zT = zpool.tile([128, NL], F32, tag="zT")
        nc.scalar.activation(zT[:], fc_ps[:], ACT.Relu,
                             bias=fcb[:], scale=1.0)
        nc.vector.tensor_scalar(zT[:], zT[:], bna[:], bnb[:],
                                op0=AL.mult, op1=AL.add)

        if kdebug == "z":
            zdbg = zpool.tile([128, NL], F16, tag="zdbg")
            nc.vector.tensor_copy(zdbg[:], zT[:])
            nc.sync.dma_start(out_d[0:128, 0:NL], zdbg[:])
            return

        # Kick off the gather of raw (uncentered) z as early as possible;
        # fp16 halves the collective bytes (local rows keep f32 precision).
        zr = zpool.tile([128, NL], F16, tag="zr")
        nc.vector.tensor_copy(zr[:], zT[:])
        nc.sync.dma_start(gin_d[0:128, :], zr[:])
        nc.gpsimd.collective_compute(
            "AllGather", AL.bypass,
            replica_groups=[list(range(N_CORES))],
            ins=[gin_d[:]], outs=[gout_d[:]],
        )

        # Center z by the local mean embedding: pairwise distances are
        # translation-invariant, and centering kills the catastrophic
        # cancellation in sq_i + sq_j - 2 z.z (embeddings cluster tightly,
        # so |z|^2 >> d2).  The same local mean is subtracted from the rows
        # AND from this core's gathered copy of the columns, so every d2
        # this core computes is exact; raw z is what gets gathered.
        zm = zpool.tile([128, 1], F32, tag="zm")
        nc.vector.tensor_reduce(zm[:], zT[:], axis=mybir.AxisListType.X,
                                op=AL.add)
        nc.scalar.mul(zm[:], zm[:], 1.0 / NL)
        zcT = zpool.tile([128, NL], F32, tag="zcT")
        nc.vector.tensor_scalar(zcT[:], zT[:], zm[:], None, op0=AL.subtract)

        zm2r = zpool.tile([128, NL], F32R, tag="zm2r")
        nc.vector.tensor_scalar_mul(zm2r[:], zcT[:], -2.0)

        # |zc|^2 row for the lhsT rank terms: ones^T (zc*zc), f32r
        zsqr = zpool.tile([128, NL], F32R, tag="zsqr")
        nc.scalar.activation(zsqr[:], zcT[:], ACT.Square)
        ones_col = zpool.tile([128, 1], F32R, tag="ones_col")
        nc.sync.dma_start(ones_col[:], onc_d[:])
        sq_ps = sqpool.tile([1, NL], F32, tag="sq")
        nc.tensor.matmul(sq_ps[:], ones_col[:], zsqr[:],
                         start=True, stop=True)
        # sqones rows: [ones ; sq_local_centered] (f32r).  The sq row sits
        # at partition 1, which DVE can't address, so it goes PSUM -> DVE ->
        # partition-0 staging -> one SBUF-to-SBUF DMA (off the critical
        # path; it only happens once).
        sqones = zpool.tile([2, NL], F32R, tag="sqones")
        nc.sync.dma_start(sqones[0:1, :], onr_d[0:1, 0:NL])
        sqrow = zpool.tile([1, NL], F32R, tag="sqrow")
        nc.vector.tensor_copy(sqrow[:], sq_ps[:])
        nc.sync.dma_start(sqones[1:2, :], sqrow[:])

        # Per-block prefetch of the gathered z (pipelines with loss tiles).
        zfT = zpool.tile([128, N_CORES, NL], F16, tag="zfT")
        for jc in range(N_CORES):
            nc.sync.dma_start(zfT[:, jc, :], gout_d[jc, :, :])
        # onesqf rows: [per-block |zc_j|^2 filled below ; ones] (f32r), so
        # the per-block DVE copy of sq_j lands on partition 0.
        onesqf = zpool.tile([2, N], F32R, tag="onesqf")
        nc.sync.dma_start(onesqf[1:2, :], onr_d[:])

        # Precompute all 32 y-mask tiles into SBUF while the AllGather is in
        # flight (they only depend on inputs), so the loss loop is free of
        # the py matmul and its dependency chain.  Indexed rb*8+jc so a
        # jc-pair for one row block is contiguous ([128, 2*NL] views).
        ym = zpool.tile([128, 32, NL], BF16, tag="ym")
        with tc.tile_pool(name="py", bufs=2, space="PSUM") as pypool:
            for jc in range(N_CORES):
                js = slice(NL * jc, NL * jc + NL)
                for rb in range(4):
                    rs = slice(128 * rb, 128 * rb + 128)
                    py = pypool.tile([128, NL], F32, tag="py")
                    nc.tensor.matmul(py[:], abl[:, rs], abf[:, js],
                                     start=True, stop=True)
                    nc.scalar.copy(ym[:, 8 * rb + jc, :], py[:])

        if kdebug == "gather":
            zfc = zpool.tile([128, NL], F16, tag="zfcd")
            nc.vector.tensor_copy(zfc[:], zfT[:, 0, :])
            nc.sync.dma_start(out_d[0:128, 0:NL], zfc[:])
            return

        # ---- loss row block ----
        # jc blocks processed in pairs: the two d2 matmuls land in the two
        # banks of one PSUM tile, then ONE wide [128, 2*NL] chain follows:
        # ACT sqrt(d2 + eps) (the +eps bias replaces the DVE clamp; d2's
        # f32r rounding can only go ~1e-4 negative), hinge relu(1-d) on ACT
        # for even tiles / DVE for odd (engine balance), DVE predicated
        # select, one DMA out.  The column prep (center, square, |z|^2) runs
        # per jc; squares go to the otherwise-idle Pool engine.
        with (
            tc.tile_pool(name="pd", bufs=2, space="PSUM") as pdpool,
            tc.tile_pool(name="sq2", bufs=2, space="PSUM") as sq2pool,
            tc.tile_pool(name="zc", bufs=4) as zcpool,
            tc.tile_pool(name="lw", bufs=4) as lwpool,
        ):
            epsb = zpool.tile([128, 1], F32, tag="epsb")
            nc.gpsimd.memset(epsb[:], 1e-3)
            for jp in range(N_CORES // 2):
                js2 = slice(2 * NL * jp, 2 * NL * jp + 2 * NL)
                zfcs = []
                for jc in (2 * jp, 2 * jp + 1):
                    js = slice(NL * jc, NL * jc + NL)
                    # center this block's columns with the local mean, then
                    # compute their squared norms
                    zfc = zcpool.tile([128, NL], F32R, tag="zfc")
                    nc.vector.tensor_scalar(zfc[:], zfT[:, jc, :], zm[:],
                                            None, op0=AL.subtract)
                    zfsq = zcpool.tile([128, NL], F32R, tag="zfsq")
                    nc.scalar.activation(zfsq[:], zfc[:], ACT.Square)
                    sq2 = sq2pool.tile([1, NL], F32, tag="sq2")
                    nc.tensor.matmul(sq2[:], ones_col[:], zfsq[:],
                                     start=True, stop=True)
                    nc.vector.tensor_copy(onesqf[0:1, js], sq2[:])
                    zfcs.append(zfc)
                for rb in range(4):
                    rs = slice(128 * rb, 128 * rb + 128)
                    pd = pdpool.tile([128, 2, NL], F32, tag="pd")
                    for b in (0, 1):
                        jsb = slice(NL * (2 * jp + b), NL * (2 * jp + b) + NL)
                        nc.tensor.matmul(pd[:, b, :], zm2r[:, rs], zfcs[b][:],
                                         start=True, stop=False)
                        nc.tensor.matmul(pd[:, b, :], sqones[:, rs],
                                         onesqf[:, jsb],
                                         start=False, stop=True)
                    # the very last tile runs as two half-width chains so
                    # the end-of-kernel serial tail is shorter
                    halves = ((0, 2),) if not (jp == 3 and rb == 3) else                         ((0, 1), (1, 2))
                    for h0, h1 in halves:
                        hw = h1 - h0
                        ymv = ym[:, 8 * rb + 2 * jp + h0:
                                 8 * rb + 2 * jp + h1, :]
                        dd = lwpool.tile([128, 2, NL], F16, tag="dd")
                        ddv = dd[:, h0:h1, :]
                        nc.scalar.activation(ddv, pd[:, h0:h1, :], ACT.Sqrt,
                                             bias=epsb[:], scale=1.0)
                        cl = lwpool.tile([128, 2, NL], F16, tag="cl")
                        clv = cl[:, h0:h1, :]
                        if (4 * jp + rb) % 2 == 0:
                            nc.scalar.activation(clv, ddv, ACT.Relu,
                                                 bias=1.0, scale=-1.0)
                        else:
                            nc.vector.tensor_scalar(clv, ddv, -1.0, 1.0,
                                                    op0=AL.mult, op1=AL.add)
                            nc.vector.tensor_scalar_max(clv, clv, 0.0)
                        nc.vector.copy_predicated(
                            clv, ymv.bitcast(mybir.dt.int16), ddv)
                        jsh = slice(NL * (2 * jp + h0), NL * (2 * jp + h1))
                        nc.sync.dma_start(out_d[rs, jsh], clv)


def _prep_inputs(samples, samples_info, conv1_w, conv1_b, conv2_w, conv2_b,
                 conv3_w, conv3_b, fc_w, fc_b, bn_gamma, bn_beta, bn_mean,
                 bn_var):
    f = np.float32
    samples = np.asarray(samples, f)
    info = np.asarray(samples_info, f)
    conv1_w = np.asarray(conv1_w, f)
    conv2_w = np.asarray(conv2_w, f)
    conv3_w = np.asarray(conv3_w, f)

    assert np.all(np.asarray(conv1_b) == 0), "conv1_b != 0 unsupported"
    assert np.all(np.asarray(conv2_b) == 0), "conv2_b != 0 unsupported"
    assert np.all(np.asarray(conv3_b) == 0), "conv3_b != 0 unsupported"

    # conv1 shifted weights, position pairs (l, l+2) packed into M=128:
    # cols 0-63 use shift s, cols 64-127 use shift s+2.  Indices 27/28 are
    # the left-only (shift 27/28) variants, 29/30 right-only (shift 0/1)
    # for pairs whose two windows land in adjacent x chunks.
    w1b = np.zeros((SIG, 128, C1), f)
    for s in range(SIG):
        w1b[s, s:s + K1, :] = conv1_w[:, 0, :].T
    w1s = np.zeros((31, 128, 128), f)
    for s in range(27):
        w1s[s, :, 0:64] = w1b[s]
        w1s[s, :, 64:128] = w1b[s + 2]
    for d in range(2):
        w1s[27 + d, :, 0:64] = w1b[27 + d]
        w1s[29 + d, :, 64:128] = w1b[d]

    # conv2 DoubleRow weight pairs.  Tap t of conv2 applied to h1 slot
    # halves: even l2 taps (t0,t1)@slot(j2-1), (t2,t3)@slot(j2), t4@slot
    # (j2+1) rows 0-63; odd l2 taps (t1,t2)@slot(j2), (t3,t4)@slot(j2+1),
    # t0@slot(j2-1) rows 64-127.  [a;b] = rows 0-63 from tap a (even h1
    # parity), rows 64-127 from tap b (odd parity).
    w2t = [conv2_w[:, :, t].T for t in range(K2)]   # [64 ic, 128 oc]
    w2e = np.zeros((2, 128, C2), f)
    w2e[0, 0:64], w2e[0, 64:128] = w2t[0], w2t[1]
    w2e[1, 0:64], w2e[1, 64:128] = w2t[2], w2t[3]
    w2o = np.zeros((2, 128, C2), f)
    w2o[0, 0:64], w2o[0, 64:128] = w2t[1], w2t[2]
    w2o[1, 0:64], w2o[1, 64:128] = w2t[3], w2t[4]
    w2l = np.zeros((128, C2), f)
    w2l[0:64] = w2t[4]      # even leftover, rows 0-63
    w2l[64:128] = w2t[0]    # odd leftover, rows 64-127

    # conv3 DoubleRow pair = taps (t0, t1); leftover = tap t2.
    w3p = np.zeros((2, 128, C3), f)
    w3p[0] = conv3_w[:, :, 0].T
    w3p[1] = conv3_w[:, :, 1].T
    w3l = conv3_w[:, :, 2].T.copy()

    fcw = np.zeros((2, 128, 128), f)
    fcwT = np.asarray(fc_w, f).T / f(T3)   # [256, 128]
    fcw[0] = fcwT[0:128, :]
    fcw[1] = fcwT[128:256, :]
    fcb = np.asarray(fc_b, f).reshape(128, 1)
    bna = (np.asarray(bn_gamma, f) /
           np.sqrt(np.asarray(bn_var, f) + f(1e-5))).reshape(128, 1)
    bnb = (np.asarray(bn_beta, f) -
           np.asarray(bn_mean, f).reshape(128) * bna[:, 0]).reshape(128, 1)

    writer, gen = info[:, 0], info[:, 1]
    assert np.all((writer == 0) | (writer == 1)), "non-binary writer id"
    a_full = (gen * (1.0 - writer)).astype(f)
    b_full = (gen * writer).astype(f)
    abf = np.stack([a_full, b_full])          # [2, N]

    import ml_dtypes
    bf = ml_dtypes.bfloat16
    f8 = ml_dtypes.float8_e4m3
    w1s_b = w1s.astype(np.float16)
    w2e_b, w2o_b, w2l_b = (w.astype(f8) for w in (w2e, w2o, w2l))
    w3p_b, w3l_b = (w.astype(f8) for w in (w3p, w3l))

    ones_col_np = np.ones((128, 1), f)
    ones_row_np = np.ones((1, N), f)

    # x transposed, padded (49 left / 50 right + tail), cut into 18
    # overlapping 128-row chunks at stride 29
    in_maps = []
    for core in range(N_CORES):
        n0 = core * NL
        xpad = np.zeros((624, NL), f)
        xpad[49:49 + L, :] = samples[n0:n0 + NL, 0, :].T
        xsc = np.zeros((NCHUNK1, 128, NL), f)
        for c in range(NCHUNK1):
            xsc[c] = xpad[SIG * c:SIG * c + 128, :]
        in_maps.append({
            "xs": xsc.astype(np.float16), "onc": ones_col_np,
            "onr": ones_row_np,
            "w1s": w1s_b, "w2e": w2e_b, "w2o": w2o_b, "w2l": w2l_b,
            "w3p": w3p_b, "w3l": w3l_b, "fcw": fcw,
            "fcb": fcb,
            "bna": bna, "bnb": bnb,
            "abl": np.ascontiguousarray(abf[:, n0:n0 + NL]).astype(bf),
            "abf": abf.astype(bf),
        })
    return in_maps


def kernel(**inputs):
    global LAST_RESULT
    in_maps = _prep_inputs(**inputs)
    nc = build_nc()
    res = run_bass_kernel_spmd(nc, in_maps, core_ids=list(range(N_CORES)))
    LAST_RESULT = res
    out = np.concatenate([r["out"] for r in res.results], axis=0)
    np.fill_diagonal(out, 0.0)
    return out.astype(np.float32)


# revision 23
# speedup vs baseline: 1.0143x; 1.0143x over previous
"""Trainium2 Bass kernel for nn_CnnModel_70007966925195.

CNN backbone (3x conv1d+relu+maxpool2 -> mean -> FC+relu -> BN) followed by an
all-pairs contrastive loss. Data-parallel over N across 8 NeuronCores; z is
AllGathered (fp16) and each core computes a 512x4096 row block of the loss.

Layout strategy:
- conv1 (C_in=1, k=100): fp16. x stored transposed+padded as overlapping
  128-row position chunks in SBUF; the tap-window select is folded into
  pre-shifted weight matrices (zero-padded to K=128). Two output positions
  (l, l+2) are packed into one M=128 matmul (cols 0-63 / 64-127), free dim =
  all 512 local samples.
- conv2/conv3: fp8(e4m3) activations+weights with DoubleRow matmuls (K=256
  per instruction: two 128-row K-halves read from adjacent ring slots).
  conv2: per output position one DR matmul (4 taps) + one K=64 leftover tap;
  the even/odd leftovers sit in disjoint row groups (0-63 / 64-127) and run
  concurrently via tile_position. conv3: DR (taps 0,1) + K=128 leftover
  (tap 2) per out-channel half. Rings carry 9 slots: slot 8 duplicates slot
  0 so DR pairs (7,0) stay contiguous. relu+maxpool eviction = ACT
  relu(bank0) -> SBUF f32, then DVE max(tmp, bank1) -> fp8 ring.
- mean+FC: h3 tiles are summed into an SBUF accumulator on the otherwise
  idle Pool engine (weights pre-scaled by 1/64); FC itself is 2 matmuls.
- loss: z AllGathered as fp16 (local rows stay f32); d2 per 128x512 tile is
  one fp32r K=128 matmul (z.z) plus one fp32r K=2 rank-2 matmul (sq_i+sq_j);
  y-mask via bf16 K=2 matmuls precomputed during the collective; then
  clamp/sqrt/relu(1-d)/select, DMA out. Pairwise distances are translation
  invariant, so rows AND gathered columns are centered by the local mean
  embedding to kill catastrophic cancellation.
"""

import os
import sys

try:
    import concourse.bass as bass  # noqa: F401
except ImportError:
    sys.path.insert(0, "/opt/trn_rl_repo")

import numpy as np

import concourse.bass as bass  # noqa: F811
import concourse.mybir as mybir
import concourse.tile as tile
from concourse import bacc
from concourse.bass_utils import run_bass_kernel_spmd

F32 = mybir.dt.float32
F32R = mybir.dt.float32r
BF16 = mybir.dt.bfloat16
F16 = mybir.dt.float16
FP8 = mybir.dt.float8e4
AL = mybir.AluOpType
ACT = mybir.ActivationFunctionType
DR = mybir.MatmulPerfMode.DoubleRow

N_CORES = 8
N = 4096
NL = N // N_CORES   # 512 samples per core
L = 512
K1, C1 = 100, 64          # conv1 kernel/outch
K2, C2 = 5, 128           # conv2
K3, C3 = 3, 256           # conv3
NCHUNK1 = 18              # conv1 x chunks, stride 29
SIG = 29                  # shift count (chunk stride)
T1 = 256                  # pooled conv1 positions
T2 = 128                  # pooled conv2 positions
T3 = 64                   # pooled conv3 positions
W1R = 8                   # h1 ring depth (slot 8 = dup of slot 0)
W2R = 8                   # h2 ring depth (slot 8 = dup of slot 0)

LAST_RESULT = None        # BassKernelResults stash for test harness


def build_nc():
    kdebug = os.environ.get("KDEBUG", "full")
    nc = bacc.Bacc("TRN2", target_bir_lowering=False, debug=False,
                   num_devices=N_CORES)

    xs_d = nc.dram_tensor("xs", [NCHUNK1, 128, NL], F16, kind="ExternalInput")
    w1s_d = nc.dram_tensor("w1s", [31, 128, 128], F16, kind="ExternalInput")
    w2e_d = nc.dram_tensor("w2e", [2, 128, C2], FP8, kind="ExternalInput")
    w2o_d = nc.dram_tensor("w2o", [2, 128, C2], FP8, kind="ExternalInput")
    w2l_d = nc.dram_tensor("w2l", [128, C2], FP8, kind="ExternalInput")
    w3p_d = nc.dram_tensor("w3p", [2, 128, C3], FP8, kind="ExternalInput")
    w3l_d = nc.dram_tensor("w3l", [128, C3], FP8, kind="ExternalInput")
    fcw_d = nc.dram_tensor("fcw", [2, 128, 128], F16, kind="ExternalInput")
    fcb_d = nc.dram_tensor("fcb", [128, 1], F32, kind="ExternalInput")
    bna_d = nc.dram_tensor("bna", [128, 1], F32, kind="ExternalInput")
    bnb_d = nc.dram_tensor("bnb", [128, 1], F32, kind="ExternalInput")
    abl_d = nc.dram_tensor("abl", [2, NL], BF16, kind="ExternalInput")
    abf_d = nc.dram_tensor("abf", [2, N], BF16, kind="ExternalInput")
    onc_d = nc.dram_tensor("onc", [128, 1], F16, kind="ExternalInput")
    onr_d = nc.dram_tensor("onr", [1, N], F16, kind="ExternalInput")
    out_d = nc.dram_tensor("out", [NL, N], F16, kind="ExternalOutput")
    gin_d = nc.dram_tensor("gin", [128, NL], F16, kind="Internal")
    gout_d = nc.dram_tensor("gout", [N_CORES, 128, NL], F16, kind="Internal",
                            addr_space="Shared")

    with tile.TileContext(nc) as tc:
        with (
            tc.tile_pool(name="const", bufs=1) as cpool,
            tc.tile_pool(name="zbuf", bufs=1) as zpool,
        ):
            # ---- persistent SBUF tensors (issue order = startup order) ----
            w1s = cpool.tile([128, 31, 128], F16, tag="w1s")
            nc.sync.dma_start(w1s[:, 0:2, :],
                              w1s_d[0:2].rearrange("s k o -> k s o"))
            xs = cpool.tile([128, NCHUNK1, NL], F16, tag="xs")
            nc.sync.dma_start(xs[:, 0, :], xs_d[0, :, :])
            nc.sync.dma_start(w1s[:, 2:8, :],
                              w1s_d[2:8].rearrange("s k o -> k s o"))
            nc.sync.dma_start(xs[:, 1, :], xs_d[1, :, :])
            nc.sync.dma_start(w1s[:, 8:31, :],
                              w1s_d[8:31].rearrange("s k o -> k s o"))
            nc.sync.dma_start(xs[:, 2:4, :],
                              xs_d[2:4].rearrange("c k n -> k c n"))
            nc.sync.dma_start(xs[:, 4:6, :],
                              xs_d[4:6].rearrange("c k n -> k c n"))
            w2e = cpool.tile([128, 2, C2], FP8, tag="w2e")
            nc.sync.dma_start(w2e[:], w2e_d[:].rearrange("t k o -> k t o"))
            w2o = cpool.tile([128, 2, C2], FP8, tag="w2o")
            nc.sync.dma_start(w2o[:], w2o_d[:].rearrange("t k o -> k t o"))
            w2l = cpool.tile([128, C2], FP8, tag="w2l")
            nc.sync.dma_start(w2l[:], w2l_d[:])
            nc.sync.dma_start(xs[:, 6:12, :],
                              xs_d[6:12].rearrange("c k n -> k c n"))
            w3p = cpool.tile([128, 2, C3], FP8, tag="w3p")
            nc.sync.dma_start(w3p[:], w3p_d[:].rearrange("t k o -> k t o"))
            w3l = cpool.tile([128, C3], FP8, tag="w3l")
            nc.sync.dma_start(w3l[:], w3l_d[:])
            nc.sync.dma_start(xs[:, 12:18, :],
                              xs_d[12:18].rearrange("c k n -> k c n"))
            fcw = cpool.tile([128, 2, 128], F16, tag="fcw")
            nc.sync.dma_start(fcw[:], fcw_d[:].rearrange("c k o -> k c o"))
            fcb = cpool.tile([128, 1], F32, tag="fcb")
            nc.sync.dma_start(fcb[:], fcb_d[:])
            bna = cpool.tile([128, 1], F32, tag="bna")
            nc.sync.dma_start(bna[:], bna_d[:])
            bnb = cpool.tile([128, 1], F32, tag="bnb")
            nc.sync.dma_start(bnb[:], bnb_d[:])
            abl = cpool.tile([2, NL], BF16, tag="abl")
            nc.sync.dma_start(abl[:], abl_d[:])
            abf = cpool.tile([2, N], BF16, tag="abf")
            nc.sync.dma_start(abf[:], abf_d[:])

            warm = cpool.tile([128, 1], F32, tag="warm")
            nc.gpsimd.memset(warm[:], 1.0)
            nc.scalar.activation(warm[:], warm[:], ACT.Sqrt)

            h1r = cpool.tile([128, W1R + 1, NL], FP8, tag="h1r")
            h2r = cpool.tile([128, W2R + 1, NL], FP8, tag="h2r")
            hsum = cpool.tile([128, 2, NL], F32, tag="hsum")
            nc.gpsimd.memset(hsum[:], 0.0)

            # ---- fused conv pipeline ----
            with (
                tc.tile_pool(name="p1", bufs=1, space="PSUM") as p1pool,
                tc.tile_pool(name="p2", bufs=1, space="PSUM") as p2pool,
                tc.tile_pool(name="p3", bufs=2, space="PSUM") as p3pool,
                tc.tile_pool(name="h3", bufs=3) as h3pool,
                tc.tile_pool(name="ev", bufs=4) as evpool,
            ):
                def conv1_batch(i1):
                    # positions l = 4*i1 .. 4*i1+3 -> pooled t = 2*i1, 2*i1+1
                    # bank j: partitions 0-63 = pos 4i+j, 64-127 = pos 4i+2+j
                    ps = p1pool.tile([128, 2, NL], F32, tag="p1")
                    for j in range(2):
                        la = 4 * i1 + j
                        lb = la + 2
                        ca, sa = divmod(la, SIG)
                        cb, sb_ = divmod(lb, SIG)
                        if ca == cb:
                            nc.tensor.matmul(
                                ps[:, j, :], w1s[:, sa, :], xs[:, ca, :],
                                start=True, stop=True)
                        else:
                            nc.tensor.matmul(
                                ps[:, j, :], w1s[:, sa, :], xs[:, ca, :],
                                start=True, stop=False)
                            nc.tensor.matmul(
                                ps[:, j, :], w1s[:, 29 + sb_, :],
                                xs[:, cb, :],
                                start=False, stop=True)
                    # eviction: DVE may read only ONE PSUM operand per op,
                    # so ACT moves relu(bank0) to SBUF first (max(relu(a),b)
                    # == relu(max(a,b)) since b is max'd with a relu output)
                    tmp = evpool.tile([128, NL], F32, tag="ev")
                    nc.scalar.activation(tmp[:], ps[:, 0, :], ACT.Relu)
                    slot = i1 % W1R
                    nc.vector.tensor_max(h1r[:, slot, :], tmp[:],
                                         ps[:, 1, :])
                    if slot == 0:
                        nc.vector.tensor_copy(h1r[:, W1R, :], h1r[:, 0, :])

                def conv2_pair(j2):
                    # output positions l2 = 2*j2 (bank0), 2*j2+1 (bank1).
                    # even: DR pair = h1 slots (j2-1, j2) with [t0;t1],[t2;t3]
                    #       leftover = tap t4 (rows 0-63) @ slot j2+1
                    # odd:  DR pair = slots (j2, j2+1) with [t1;t2],[t3;t4]
                    #       leftover = tap t0 (rows 64-127) @ slot j2-1
                    # Each bank's matmul list is built first so start/stop
                    # land on the true first/last op of the group; the two
                    # leftovers occupy disjoint row groups (0-63 / 64-127)
                    # and are emitted adjacently so they run concurrently.
                    ps = p2pool.tile([128, 2, NL], F32, tag="p2")
                    main, left = [], []
                    if j2 >= 1:
                        main.append((0, "dr", w2e[:], (j2 - 1) % W1R))
                    else:        # l2 = 0: only taps t2,t3 @ slot 0
                        main.append((0, "n", w2e[:, 1, :], 0))
                    if j2 + 1 < T1 // 2:
                        left.append((0, 0, (j2 + 1) % W1R))
                    if j2 + 1 < T1 // 2:
                        main.append((1, "dr", w2o[:], j2 % W1R))
                    else:        # l2 = 255: only taps t1,t2 @ slot 127
                        main.append((1, "n", w2o[:, 0, :], j2 % W1R))
                    if j2 >= 1:
                        left.append((1, 64, (j2 - 1) % W1R))
                    has_left = {b for b, _, _ in left}
                    for bank, kind, w, slot in main:
                        stop = bank not in has_left
                        if kind == "dr":
                            nc.tensor.matmul(ps[:, bank, :], w,
                                             h1r[:, slot:slot + 2, :],
                                             start=True, stop=stop,
                                             perf_mode=DR)
                        else:
                            nc.tensor.matmul(ps[:, bank, :], w,
                                             h1r[:, slot, :],
                                             start=True, stop=stop)
                    for bank, rb, slot in left:
                        nc.tensor.matmul(
                            ps[:, bank, :], w2l[rb:rb + 64, :],
                            h1r[rb:rb + 64, slot, :],
                            start=False, stop=True,
                            tile_position=(rb, 0))
                    tmp = evpool.tile([128, NL], F32, tag="ev")
                    nc.scalar.activation(tmp[:], ps[:, 0, :], ACT.Relu)
                    slot = j2 % W2R
                    nc.vector.tensor_max(h2r[:, slot, :], tmp[:],
                                         ps[:, 1, :])
                    if slot == 0:
                        nc.vector.tensor_copy(h2r[:, W2R, :], h2r[:, 0, :])

                def conv3_half(j3, ch):
                    # output positions l3 = 2*j3 (bank0), 2*j3+1 (bank1),
                    # out-channel half ch.  DR pair = h2 slots (l3-1, l3)
                    # with taps (t0, t1); leftover = tap t2 @ slot l3+1.
                    ps = p3pool.tile([128, 2, NL], F32, tag="p3")
                    cs = slice(128 * ch, 128 * ch + 128)
                    for jj in range(2):
                        l3 = 2 * j3 + jj
                        has_left = l3 + 1 < T2
                        if l3 >= 1:
                            a = (l3 - 1) % W2R
                            nc.tensor.matmul(ps[:, jj, :], w3p[:, :, cs],
                                             h2r[:, a:a + 2, :],
                                             start=True, stop=not has_left,
                                             perf_mode=DR)
                        else:
                            # l3 = 0: tap t1 @ slot 0 only
                            nc.tensor.matmul(ps[:, jj, :], w3p[:, 1, cs],
                                             h2r[:, 0, :],
                                             start=True, stop=not has_left)
                        if has_left:
                            nc.tensor.matmul(ps[:, jj, :], w3l[:, cs],
                                             h2r[:, (l3 + 1) % W2R, :],
                                             start=False, stop=True)
                    h3t = h3pool.tile([128, NL], F16, tag="h3t")
                    tmp = evpool.tile([128, NL], F32, tag="ev")
                    nc.scalar.activation(tmp[:], ps[:, 0, :], ACT.Relu)
                    nc.vector.tensor_max(h3t[:], tmp[:], ps[:, 1, :])
                    nc.gpsimd.tensor_tensor(hsum[:, ch, :], hsum[:, ch, :],
                                            h3t[:], AL.add)

                for ii in range(132):
                    if ii < 128:
                        conv1_batch(ii)
                    if 2 <= ii < 130:
                        conv2_pair(ii - 2)
                    if ii >= 4:
                        j3, ch = divmod(ii - 4, 2)
                        if j3 < T3:
                            conv3_half(j3, ch)

            _emit_tail(nc, tc, zpool, hsum, fcw, fcb, bna, bnb, abl, abf,
                       onc_d, onr_d, out_d, gin_d, gout_d, kdebug)

    nc.compile()
    return nc


def _emit_tail(nc, tc, zpool, hsum, fcw, fcb, bna, bnb, abl, abf,
               onc_d, onr_d, out_d, gin_d, gout_d, kdebug):
    # ---- FC from pooled means, z = BN(relu(FC)) ; gather z + |z|^2 ----
    with (
        tc.tile_pool(name="fcp", bufs=1, space="PSUM") as fcpool,
        tc.tile_pool(name="sqp", bufs=1, space="PSUM") as sqpool,
    ):
        hsb = zpool.tile([128, 2, NL], F16, tag="hsb")
        nc.vector.tensor_copy(hsb[:], hsum[:])
        fc_ps = fcpool.tile([128, NL], F32, tag="fc")
        nc.tensor.matmul(fc_ps[:], fcw[:, 0, :], hsb[:, 0, :],
                         start=True, stop=False)
        nc.tensor.matmul(fc_ps[:], fcw[:, 1, :], hsb[:, 1, :],
                         start=False, stop=True)

        zT = zpool.tile([128, NL], F32, tag="zT")
        nc.scalar.activation(zT[:], fc_ps[:], ACT.Relu,
                             bias=fcb[:], scale=1.0)
        nc.vector.tensor_scalar(zT[:], zT[:], bna[:], bnb[:],
                                op0=AL.mult, op1=AL.add)

        if kdebug == "z":
            zdbg = zpool.tile([128, NL], F16, tag="zdbg")
            nc.vector.tensor_copy(zdbg[:], zT[:])
            nc.sync.dma_start(out_d[0:128, 0:NL], zdbg[:])
            return

        # Kick off the gather of raw (uncentered) z as early as possible;
        # fp16 halves the collective bytes (local rows keep f32 precision).
        zr = zpool.tile([128, NL], F16, tag="zr")
        nc.vector.tensor_copy(zr[:], zT[:])
        nc.sync.dma_start(gin_d[0:128, :], zr[:])
        nc.gpsimd.collective_compute(
            "AllGather", AL.bypass,
            replica_groups=[list(range(N_CORES))],
            ins=[gin_d[:]], outs=[gout_d[:]],
        )

        # Center z by the local mean embedding: pairwise distances are
        # translation-invariant, and centering kills the catastrophic
        # cancellation in sq_i + sq_j - 2 z.z (embeddings cluster tightly,
        # so |z|^2 >> d2).  The same local mean is subtracted from the rows
        # AND from this core's gathered copy of the columns, so every d2
        # this core computes is exact; raw z is what gets gathered.
        zm = zpool.tile([128, 1], F32, tag="zm")
        nc.vector.tensor_reduce(zm[:], zT[:], axis=mybir.AxisListType.X,
                                op=AL.add)
        nc.scalar.mul(zm[:], zm[:], 1.0 / NL)
        zcT = zpool.tile([128, NL], F32, tag="zcT")
        nc.vector.tensor_scalar(zcT[:], zT[:], zm[:], None, op0=AL.subtract)

        zm2r = zpool.tile([128, NL], F16, tag="zm2r")
        nc.vector.tensor_scalar_mul(zm2r[:], zcT[:], -2.0)

        # |zc|^2 row for the lhsT rank terms: ones^T (zc*zc), f32r
        zsqr = zpool.tile([128, NL], F16, tag="zsqr")
        nc.scalar.activation(zsqr[:], zcT[:], ACT.Square)
        ones_col = zpool.tile([128, 1], F16, tag="ones_col")
        nc.sync.dma_start(ones_col[:], onc_d[:])
        sq_ps = sqpool.tile([1, NL], F32, tag="sq")
        nc.tensor.matmul(sq_ps[:], ones_col[:], zsqr[:],
                         start=True, stop=True)
        # sqones rows: [ones ; sq_local_centered] (f32r).  The sq row sits
        # at partition 1, which DVE can't address, so it goes PSUM -> DVE ->
        # partition-0 staging -> one SBUF-to-SBUF DMA (off the critical
        # path; it only happens once).
        sqones = zpool.tile([2, NL], F16, tag="sqones")
        nc.sync.dma_start(sqones[0:1, :], onr_d[0:1, 0:NL])
        sqrow = zpool.tile([1, NL], F16, tag="sqrow")
        nc.vector.tensor_copy(sqrow[:], sq_ps[:])
        nc.sync.dma_start(sqones[1:2, :], sqrow[:])

        # Gathered z fetched in two chunks: the first jc-pair's 256KB
        # lands quickly so loss work starts, the rest streams behind it.
        zfT = zpool.tile([128, N_CORES, NL], F16, tag="zfT")
        nc.sync.dma_start(zfT[:, 0:2, :],
                          gout_d[0:2].rearrange("r p n -> p r n"))
        nc.sync.dma_start(zfT[:, 2:8, :],
                          gout_d[2:8].rearrange("r p n -> p r n"))
        # onesqf rows: [per-block |zc_j|^2 filled below ; ones] (f32r), so
        # the per-block DVE copy of sq_j lands on partition 0.
        onesqf = zpool.tile([2, N], F16, tag="onesqf")
        nc.sync.dma_start(onesqf[1:2, :], onr_d[:])

        # Precompute all 32 y-mask tiles into SBUF while the AllGather is in
        # flight (they only depend on inputs), so the loss loop is free of
        # the py matmul and its dependency chain.  Indexed rb*8+jc so a
        # jc-pair for one row block is contiguous ([128, 2*NL] views).
        ym = zpool.tile([128, 32, NL], BF16, tag="ym")
        with tc.tile_pool(name="py", bufs=2, space="PSUM") as pypool:
            for jc in range(N_CORES):
                js = slice(NL * jc, NL * jc + NL)
                for rb in range(4):
                    rs = slice(128 * rb, 128 * rb + 128)
                    py = pypool.tile([128, NL], F32, tag="py")
                    nc.tensor.matmul(py[:], abl[:, rs], abf[:, js],
                                     start=True, stop=True)
                    nc.scalar.copy(ym[:, 8 * rb + jc, :], py[:])

        if kdebug == "gather":
            zfc = zpool.tile([128, NL], F16, tag="zfcd")
            nc.vector.tensor_copy(zfc[:], zfT[:, 0, :])
            nc.sync.dma_start(out_d[0:128, 0:NL], zfc[:])
            return

        # ---- loss row block ----
        # jc blocks processed in pairs: the two d2 matmuls land in the two
        # banks of one PSUM tile, then ONE wide [128, 2*NL] chain follows:
        # ACT sqrt(d2 + eps) (the +eps bias replaces the DVE clamp; d2's
        # f32r rounding can only go ~1e-4 negative), hinge relu(1-d) on ACT
        # for even tiles / DVE for odd (engine balance), DVE predicated
        # select, one DMA out.  The column prep (center, square, |z|^2) runs
        # per jc; squares go to the otherwise-idle Pool engine.
        with (
            tc.tile_pool(name="pd", bufs=2, space="PSUM") as pdpool,
            tc.tile_pool(name="sq2", bufs=2, space="PSUM") as sq2pool,
            tc.tile_pool(name="zc", bufs=4) as zcpool,
            tc.tile_pool(name="lw", bufs=4) as lwpool,
        ):
            epsb = zpool.tile([128, 1], F32, tag="epsb")
            nc.gpsimd.memset(epsb[:], 1e-3)
            for jp in range(N_CORES // 2):
                js2 = slice(2 * NL * jp, 2 * NL * jp + 2 * NL)
                zfcs = []
                for jc in (2 * jp, 2 * jp + 1):
                    js = slice(NL * jc, NL * jc + NL)
                    # center this block's columns with the local mean, then
                    # compute their squared norms
                    zfc = zcpool.tile([128, NL], F16, tag="zfc")
                    nc.vector.tensor_scalar(zfc[:], zfT[:, jc, :], zm[:],
                                            None, op0=AL.subtract)
                    zfsq = zcpool.tile([128, NL], F16, tag="zfsq")
                    nc.scalar.activation(zfsq[:], zfc[:], ACT.Square)
                    sq2 = sq2pool.tile([1, NL], F32, tag="sq2")
                    nc.tensor.matmul(sq2[:], ones_col[:], zfsq[:],
                                     start=True, stop=True)
                    nc.vector.tensor_copy(onesqf[0:1, js], sq2[:])
                    zfcs.append(zfc)
                for rb in range(4):
                    rs = slice(128 * rb, 128 * rb + 128)
                    pd = pdpool.tile([128, 2, NL], F32, tag="pd")
                    for b in (0, 1):
                        jsb = slice(NL * (2 * jp + b), NL * (2 * jp + b) + NL)
                        nc.tensor.matmul(pd[:, b, :], zm2r[:, rs], zfcs[b][:],
                                         start=True, stop=False)
                        nc.tensor.matmul(pd[:, b, :], sqones[:, rs],
                                         onesqf[:, jsb],
                                         start=False, stop=True)
                    # the very last tile runs as two half-width chains so
                    # the end-of-kernel serial tail is shorter
                    halves = ((0, 2),) if not (jp == 3 and rb == 3) else                         ((0, 1), (1, 2))
                    for h0, h1 in halves:
                        hw = h1 - h0
                        ymv = ym[:, 8 * rb + 2 * jp + h0:
                                 8 * rb + 2 * jp + h1, :]
                        dd = lwpool.tile([128, 2, NL], F16, tag="dd")
                        ddv = dd[:, h0:h1, :]
                        nc.scalar.activation(ddv, pd[:, h0:h1, :], ACT.Sqrt,
                                             bias=epsb[:], scale=1.0)
                        cl = lwpool.tile([128, 2, NL], F16, tag="cl")
                        clv = cl[:, h0:h1, :]
                        if (4 * jp + rb) % 2 == 0:
                            nc.scalar.activation(clv, ddv, ACT.Relu,
                                                 bias=1.0, scale=-1.0)
                        else:
                            nc.vector.tensor_scalar(clv, ddv, -1.0, 1.0,
                                                    op0=AL.mult, op1=AL.add)
                            nc.vector.tensor_scalar_max(clv, clv, 0.0)
                        nc.vector.copy_predicated(
                            clv, ymv.bitcast(mybir.dt.int16), ddv)
                        jsh = slice(NL * (2 * jp + h0), NL * (2 * jp + h1))
                        nc.sync.dma_start(out_d[rs, jsh], clv)


def _prep_inputs(samples, samples_info, conv1_w, conv1_b, conv2_w, conv2_b,
                 conv3_w, conv3_b, fc_w, fc_b, bn_gamma, bn_beta, bn_mean,
                 bn_var):
    f = np.float32
    samples = np.asarray(samples, f)
    info = np.asarray(samples_info, f)
    conv1_w = np.asarray(conv1_w, f)
    conv2_w = np.asarray(conv2_w, f)
    conv3_w = np.asarray(conv3_w, f)

    assert np.all(np.asarray(conv1_b) == 0), "conv1_b != 0 unsupported"
    assert np.all(np.asarray(conv2_b) == 0), "conv2_b != 0 unsupported"
    assert np.all(np.asarray(conv3_b) == 0), "conv3_b != 0 unsupported"

    # conv1 shifted weights, position pairs (l, l+2) packed into M=128:
    # cols 0-63 use shift s, cols 64-127 use shift s+2.  Indices 27/28 are
    # the left-only (shift 27/28) variants, 29/30 right-only (shift 0/1)
    # for pairs whose two windows land in adjacent x chunks.
    w1b = np.zeros((SIG, 128, C1), f)
    for s in range(SIG):
        w1b[s, s:s + K1, :] = conv1_w[:, 0, :].T
    w1s = np.zeros((31, 128, 128), f)
    for s in range(27):
        w1s[s, :, 0:64] = w1b[s]
        w1s[s, :, 64:128] = w1b[s + 2]
    for d in range(2):
        w1s[27 + d, :, 0:64] = w1b[27 + d]
        w1s[29 + d, :, 64:128] = w1b[d]

    # conv2 DoubleRow weight pairs.  Tap t of conv2 applied to h1 slot
    # halves: even l2 taps (t0,t1)@slot(j2-1), (t2,t3)@slot(j2), t4@slot
    # (j2+1) rows 0-63; odd l2 taps (t1,t2)@slot(j2), (t3,t4)@slot(j2+1),
    # t0@slot(j2-1) rows 64-127.  [a;b] = rows 0-63 from tap a (even h1
    # parity), rows 64-127 from tap b (odd parity).
    w2t = [conv2_w[:, :, t].T for t in range(K2)]   # [64 ic, 128 oc]
    w2e = np.zeros((2, 128, C2), f)
    w2e[0, 0:64], w2e[0, 64:128] = w2t[0], w2t[1]
    w2e[1, 0:64], w2e[1, 64:128] = w2t[2], w2t[3]
    w2o = np.zeros((2, 128, C2), f)
    w2o[0, 0:64], w2o[0, 64:128] = w2t[1], w2t[2]
    w2o[1, 0:64], w2o[1, 64:128] = w2t[3], w2t[4]
    w2l = np.zeros((128, C2), f)
    w2l[0:64] = w2t[4]      # even leftover, rows 0-63
    w2l[64:128] = w2t[0]    # odd leftover, rows 64-127

    # conv3 DoubleRow pair = taps (t0, t1); leftover = tap t2.
    w3p = np.zeros((2, 128, C3), f)
    w3p[0] = conv3_w[:, :, 0].T
    w3p[1] = conv3_w[:, :, 1].T
    w3l = conv3_w[:, :, 2].T.copy()

    fcw = np.zeros((2, 128, 128), f)
    fcwT = np.asarray(fc_w, f).T / f(T3)   # [256, 128]
    fcw[0] = fcwT[0:128, :]
    fcw[1] = fcwT[128:256, :]
    fcb = np.asarray(fc_b, f).reshape(128, 1)
    bna = (np.asarray(bn_gamma, f) /
           np.sqrt(np.asarray(bn_var, f) + f(1e-5))).reshape(128, 1)
    bnb = (np.asarray(bn_beta, f) -
           np.asarray(bn_mean, f).reshape(128) * bna[:, 0]).reshape(128, 1)

    writer, gen = info[:, 0], info[:, 1]
    assert np.all((writer == 0) | (writer == 1)), "non-binary writer id"
    a_full = (gen * (1.0 - writer)).astype(f)
    b_full = (gen * writer).astype(f)
    abf = np.stack([a_full, b_full])          # [2, N]

    import ml_dtypes
    bf = ml_dtypes.bfloat16
    f8 = ml_dtypes.float8_e4m3
    w1s_b = w1s.astype(np.float16)
    w2e_b, w2o_b, w2l_b = (w.astype(f8) for w in (w2e, w2o, w2l))
    w3p_b, w3l_b = (w.astype(f8) for w in (w3p, w3l))

    ones_col_np = np.ones((128, 1), np.float16)
    ones_row_np = np.ones((1, N), np.float16)

    # x transposed, padded (49 left / 50 right + tail), cut into 18
    # overlapping 128-row chunks at stride 29
    in_maps = []
    for core in range(N_CORES):
        n0 = core * NL
        xpad = np.zeros((624, NL), f)
        xpad[49:49 + L, :] = samples[n0:n0 + NL, 0, :].T
        xsc = np.zeros((NCHUNK1, 128, NL), f)
        for c in range(NCHUNK1):
            xsc[c] = xpad[SIG * c:SIG * c + 128, :]
        in_maps.append({
            "xs": xsc.astype(np.float16), "onc": ones_col_np,
            "onr": ones_row_np,
            "w1s": w1s_b, "w2e": w2e_b, "w2o": w2o_b, "w2l": w2l_b,
            "w3p": w3p_b, "w3l": w3l_b, "fcw": fcw.astype(np.float16),
            "fcb": fcb,
            "bna": bna, "bnb": bnb,
            "abl": np.ascontiguousarray(abf[:, n0:n0 + NL]).astype(bf),
            "abf": abf.astype(bf),
        })
    return in_maps


def kernel(**inputs):
    global LAST_RESULT
    in_maps = _prep_inputs(**inputs)
    nc = build_nc()
    res = run_bass_kernel_spmd(nc, in_maps, core_ids=list(range(N_CORES)))
    LAST_RESULT = res
    out = np.concatenate([r["out"] for r in res.results], axis=0)
    np.fill_diagonal(out, 0.0)
    return out.astype(np.float32)
